# revision 1
# baseline (speedup 1.0000x reference)
"""Trainium2 Bass kernel for nn_Communication_89464168775793.

Reference computation:
    a = LayerNorm(s1 @ W1.T + b1) * gamma + beta          # [512, 128]
    b = LayerNorm(s2 @ W2.T + b2) * gamma + beta          # [512, 128]
    cmct[n, m] = concat(a[n], b[m]) / (eps + N*M)         # [512, 512, 256]
    out = cmct @ Wout.T + bout                            # [1, 512, 512, 128]

Key algebra: the output projection distributes over the concat, so
    out[n, m, z] = (pa[n, z] + pb[m, z] + bias[z]) / D
with pa = LN(s1@W1.T+b1)@(Wout[:, :128]*gamma).T (LN folded as a
post-matmul per-row affine), pb the analogue for s2, and D = eps + N*M.
The [512, 512, 128] output (134 MB fp32) is a broadcast-add of two small
[512, 128] tensors -> the kernel is output-DMA-bound.

Sharding: N (rows of the pairwise grid) split across 8 cores; each core
holds s1[n-slice] plus full s2 and weights, and writes its
[64, 512, 128] block of the output.

Implementation notes:
  - Main loop per output row n: psum <- I @ pb_all (copy) accumulated with
    ones^T (x) pa[n] (partition broadcast), both as float32r matmuls
    (full-rate fp32 on the PE); ACT/DVE copy psum -> SBUF stage; 2 MB
    HWDGE DMA per 8 rows, fully contiguous in DRAM via the m = 4p + c
    partition-major interleave.
  - pa rows are replicated to one partition via a tiny DRAM round-trip.
  - pb is computed transposed (wide N=512 matmuls straight from s2T and
    the precomputed weight product W2W = W2-contracted Wout half), then
    transposed back per 128-chunk with the per-row 1/(D*std) scale applied
    on the PSUM->SBUF copy.
  - LN statistics use matmul column/row sums; the mean comes linearly from
    s2T (bias b2 handled via constant rows), variance via ACT Square.
"""

import sys

if "/opt/trn_rl_repo" not in sys.path:
    sys.path.insert(0, "/opt/trn_rl_repo")

from contextlib import ExitStack

import numpy as np

import concourse.bass as bass
import concourse.mybir as mybir
import concourse.tile as tile
from concourse import bacc, masks
from concourse.bass_utils import run_bass_kernel_spmd

F32 = mybir.dt.float32

N_CORES = 8
N_FULL = 512          # rows of pairwise grid
M_FULL = 512          # cols of pairwise grid
NSH = N_FULL // N_CORES  # 64 rows per core
C_S, C_H, C_Z = 384, 128, 128
D = 0.001 + float(N_FULL * M_FULL)   # norm divisor (module eps + n*m)
LN_EPS = 1e-5
SUP = 4               # n-rows per output staging tile / DMA (2 MB each)
MMG = 4               # psum tiles per ldweights group in the main loop


def _build_program(bench_loops: int = 1) -> bass.Bass:
    nc = bacc.Bacc("TRN2", target_bir_lowering=False)

    s1c = nc.declare_dram_parameter("s1c", [NSH, C_S], F32, isOutput=False)
    s2 = nc.declare_dram_parameter("s2", [M_FULL, C_S], F32, isOutput=False)
    W1 = nc.declare_dram_parameter("W1", [C_H, C_S], F32, isOutput=False)
    W2 = nc.declare_dram_parameter("W2", [C_H, C_S], F32, isOutput=False)
    Wout = nc.declare_dram_parameter("Wout", [C_Z, 2 * C_H], F32, isOutput=False)
    # packed rows: [b1 | b2 | gamma | beta | bout] along the free dim
    vecsp = nc.declare_dram_parameter("vecs", [1, 5 * 128], F32, isOutput=False)
    out = nc.declare_dram_parameter("out", [NSH, M_FULL, C_Z], F32, isOutput=True)

    # DRAM view for the staged stores: m = 4*p + c (partition-major), so each
    # partition's (c, z) span is 512 contiguous fp32 in DRAM.
    out_r = out[:].rearrange("n (p c) z -> p n (c z)", p=128, c=4)

    with tile.TileContext(nc) as tc, ExitStack() as ctx:
        const = ctx.enter_context(tc.tile_pool(name="const", bufs=1))
        wpool = ctx.enter_context(tc.tile_pool(name="wpool", bufs=1))
        work = ctx.enter_context(tc.tile_pool(name="work", bufs=1))
        small = ctx.enter_context(tc.tile_pool(name="small", bufs=1))
        stage_pool = ctx.enter_context(tc.tile_pool(name="stage", bufs=3))
        drampool = ctx.enter_context(tc.tile_pool(name="dram", bufs=1, space="DRAM"))
        pspool = ctx.enter_context(tc.tile_pool(name="ps", bufs=3, space="PSUM"))
        psout = ctx.enter_context(tc.tile_pool(name="psout", bufs=5, space="PSUM"))

        DD = float(D)
        F32R = mybir.dt.float32r

        # ---------------- input loads (balanced across HWDGE rings) --------
        s2_re = s2[:].rearrange("(q four) s -> four q s", four=4)
        vecs = const.tile([1, 5 * 128], F32)
        nc.scalar.dma_start(vecs[:], vecsp[:])
        s2s = []
        for c in range(4):
            t = wpool.tile([128, C_S], F32, tag=f"s2_{c}")
            (nc.sync if c < 2 else nc.scalar).dma_start(t[:], s2_re[c])
            s2s.append(t)
        W2s = wpool.tile([C_H, C_S], F32)
        nc.sync.dma_start(W2s[:], W2[:])
        Wouts = wpool.tile([C_Z, 2 * C_H], F32)
        nc.scalar.dma_start(Wouts[:], Wout[:])
        s1s = wpool.tile([NSH, C_S], F32)
        nc.gpsimd.dma_start(s1s[:], s1c[:])
        W1s = wpool.tile([C_H, C_S], F32)
        nc.gpsimd.dma_start(W1s[:], W1[:])
        b1r = vecs[0:1, 0:128]
        b2r = vecs[0:1, 128:256]
        gammar = vecs[0:1, 256:384]
        betar = vecs[0:1, 384:512]
        boutr = vecs[0:1, 512:640]

        # ---------------- constants ----------------
        I128 = const.tile([128, 128], F32)
        masks.make_identity(nc, I128[:])
        I128r = const.tile([128, 128], F32R)
        nc.vector.tensor_copy(I128r[:], I128[:])
        ones_row = const.tile([1, 512], F32)
        nc.gpsimd.memset(ones_row[:], 1.0)
        ones_row_r = const.tile([1, 512], F32R)
        nc.vector.tensor_copy(ones_row_r[:], ones_row[:])
        ones_col = const.tile([128, 1], F32)
        nc.gpsimd.memset(ones_col[:], 1.0)
        ones_col_r = const.tile([128, 1], F32R)
        nc.vector.tensor_copy(ones_col_r[:], ones_col[:])
        # eps column for the Sqrt bias: D^2 * ln_eps
        epsD2 = const.tile([128, 1], F32)
        nc.gpsimd.memset(epsD2[:], DD * DD * float(LN_EPS))
        # warm the ACT Sqrt function table before it lands on the critical path
        warm = small.tile([1, 1], F32, tag="warm")
        nc.scalar.activation(warm[:], ones_col[0:1, 0:1],
                             mybir.ActivationFunctionType.Sqrt)

        # ============ wave 1: all PE transposes (s2 first) ============
        # s2T[cs]: [s-part, (c*128 + q)] with m = 4q + c
        s2T = wpool.tile([128, 3 * 512], F32R)
        for cs in range(3):
            pst = pspool.tile([128, 512], F32, tag="ps")
            for c in range(4):
                nc.tensor.transpose(pst[:, c * 128:(c + 1) * 128],
                                    s2s[c][:, cs * 128:(cs + 1) * 128], I128[:])
            if cs == 0:
                nc.vector.tensor_copy(s2T[:, cs * 512:(cs + 1) * 512], pst[:])
            else:
                nc.scalar.copy(s2T[:, cs * 512:(cs + 1) * 512], pst[:])

        # W2^T for the variance matmuls
        W2T = wpool.tile([128, C_S], F32R)
        pstw2 = pspool.tile([128, 512], F32, tag="ps")
        for cs in range(3):
            nc.tensor.transpose(pstw2[:, cs * 128:(cs + 1) * 128],
                                W2s[:, cs * 128:(cs + 1) * 128], I128[:])
        nc.scalar.copy(W2T[:], pstw2[:, 0:384])

        # Wout^T halves + gamma/beta/b2 columns (batched into one psum)
        WABTr = wpool.tile([128, 256], F32R)  # [h, z]: WA^T | WB^T
        pstw3 = pspool.tile([128, 512], F32, tag="ps")
        for ch in range(2):
            nc.tensor.transpose(pstw3[:, ch * 128:(ch + 1) * 128],
                                Wouts[:, ch * 128:(ch + 1) * 128], I128[:])
        nc.tensor.transpose(pstw3[:, 256:257], gammar, I128[0:1, 0:1])
        nc.tensor.transpose(pstw3[:, 257:258], betar, I128[0:1, 0:1])
        nc.tensor.transpose(pstw3[:, 258:259], b2r, I128[0:1, 0:1])
        nc.vector.tensor_copy(WABTr[:], pstw3[:, 0:256])
        gb_cols = const.tile([128, 3], F32R)
        nc.vector.tensor_copy(gb_cols[:], pstw3[:, 256:259])
        gamma_colf = const.tile([128, 1], F32)
        nc.scalar.copy(gamma_colf[:], pstw3[:, 256:257])
        gamma_col = gamma_colf[:, 0:1]
        beta_col = gb_cols[:, 1:2]
        b2_col = gb_cols[:, 2:3]

        # a-side transposes
        W1T = wpool.tile([128, C_S], F32R)
        pstw = pspool.tile([128, 512], F32, tag="ps")
        for cs in range(3):
            nc.tensor.transpose(pstw[:, cs * 128:(cs + 1) * 128],
                                W1s[:, cs * 128:(cs + 1) * 128], I128[:])
        nc.scalar.copy(W1T[:], pstw[:, 0:384])
        s1T = wpool.tile([128, 3 * NSH], F32R)  # chunk cs: [s-part, n]
        psta = pspool.tile([128, 3 * NSH], F32, tag="ps")
        for cs in range(3):
            nc.tensor.transpose(psta[:, cs * NSH:(cs + 1) * NSH],
                                s1s[:, cs * 128:(cs + 1) * 128],
                                I128[0:NSH, 0:NSH])
        nc.vector.tensor_copy(s1T[:], psta[:])

        # ============ wave 2: weights-only products and rows ============
        vecs_r = const.tile([1, 5 * 128], F32R)
        nc.vector.tensor_copy(vecs_r[:], vecs[:])
        b1rr = vecs_r[0:1, 0:128]

        WABT = wpool.tile([128, 256], F32R)
        nc.scalar.mul(WABT[:], WABTr[:], gamma_col)
        W2sr = wpool.tile([C_H, C_S], F32R)
        nc.vector.tensor_copy(W2sr[:], W2s[:])

        # W2W[s, z] = sum_h W2[h, s] * WB'[h, z]
        W2W = wpool.tile([128, 3 * 128], F32R)
        psww = pspool.tile([128, 512], F32, tag="ps")
        for cs in range(3):
            nc.tensor.matmul(psww[:, cs * 128:(cs + 1) * 128],
                             lhsT=W2sr[:, cs * 128:(cs + 1) * 128],
                             rhs=WABT[:, 128:256],
                             start=True, stop=True, skip_group_check=True)
        nc.scalar.copy(W2W[:], psww[:, 0:384])

        # rows: [wsum_a | wsum_b | bias_row(beta@W + D*bout)], w2colsum, b2WB
        boutD = const.tile([1, 128], F32R)
        nc.scalar.mul(boutD[:], boutr, DD)
        psw = pspool.tile([1, 512], F32, tag="ps")
        nc.tensor.matmul(psw[0:1, 0:256], lhsT=ones_col_r[:], rhs=WABT[:],
                         start=True, stop=True)
        nc.tensor.matmul(psw[0:1, 256:384], lhsT=beta_col, rhs=WABTr[:, 0:128],
                         start=True, stop=False, skip_group_check=True)
        nc.tensor.matmul(psw[0:1, 256:384], lhsT=beta_col, rhs=WABTr[:, 128:256],
                         start=False, stop=False, skip_group_check=True)
        nc.tensor.matmul(psw[0:1, 256:384], lhsT=ones_row_r[0:1, 0:1], rhs=boutD[:],
                         start=False, stop=True, skip_group_check=True)
        # w2colsum as columns [s-chunk, cs], straight from natural W2
        psw2 = pspool.tile([128, 3], F32, tag="ps")
        for cs in range(3):
            nc.tensor.matmul(psw2[:, cs:cs + 1],
                             lhsT=W2s[:, cs * 128:(cs + 1) * 128],
                             rhs=ones_col[:], start=True, stop=True,
                             skip_group_check=True)
        w2c = const.tile([128, 4], F32R)
        nc.vector.tensor_copy(w2c[:, 0:3], psw2[:])
        b2sum_t = const.tile([1, 1], F32)
        nc.vector.tensor_reduce(b2sum_t[:], b2r, mybir.AxisListType.X,
                                mybir.AluOpType.add)
        psw3 = pspool.tile([1, 128], F32, tag="ps")
        nc.tensor.matmul(psw3[:], lhsT=b2_col, rhs=WABT[:, 128:256],
                         start=True, stop=True)

        wsum_bias = const.tile([1, 384], F32R)  # [wsum_a | wsum_b | bias_row]
        nc.vector.tensor_copy(wsum_bias[:], psw[0:1, 0:384])
        bias_dD = const.tile([1, 128], F32R)
        nc.scalar.mul(bias_dD[:], wsum_bias[0:1, 256:384], 1.0 / DD)
        b2wb_row = const.tile([1, 128], F32R)
        nc.vector.tensor_copy(b2wb_row[:], psw3[:])
        # constant row folding b2 back in: b2fix = b2@WB' - (sum(b2)/128)*wsum_b
        b2fix_row = const.tile([1, 128], F32R)
        nc.scalar.activation(b2fix_row[:], wsum_bias[0:1, 128:256],
                             mybir.ActivationFunctionType.Copy,
                             scale=-1.0 / C_H)
        nc.vector.tensor_scalar(b2fix_row[:], b2fix_row[:],
                                b2sum_t[0:1, 0:1], None,
                                mybir.AluOpType.mult)
        nc.vector.tensor_add(b2fix_row[:], b2fix_row[:], b2wb_row[:])

        # ============ wave 3: pre-LN activations + squares ============
        psb_ = pspool.tile([128, 512], F32, tag="ps")
        for cs in range(3):
            nc.tensor.matmul(psb_[:], lhsT=W2T[:, cs * 128:(cs + 1) * 128],
                             rhs=s2T[:, cs * 512:(cs + 1) * 512],
                             start=(cs == 0), stop=(cs == 2))
        sqb = work.tile([128, 512], F32R, tag="sqb")
        nc.scalar.square(sqb[:], psb_[:])

        psa = pspool.tile([128, NSH], F32, tag="ps")
        for cs in range(3):
            nc.tensor.matmul(psa[:], lhsT=W1T[:, cs * 128:(cs + 1) * 128],
                             rhs=s1T[:, cs * NSH:(cs + 1) * NSH],
                             start=(cs == 0), stop=False)
        nc.tensor.matmul(psa[:], lhsT=b1rr, rhs=ones_row_r[0:1, 0:NSH],
                         start=False, stop=True, skip_group_check=True)
        apreT = work.tile([128, NSH], F32R, tag="apreT")
        nc.vector.tensor_copy(apreT[:], psa[:])
        sqa = work.tile([128, NSH], F32R, tag="sqa")
        nc.scalar.square(sqa[:], psa[:])

        # ============ wave 4: statistic sums (PE) ============
        # b-side: mu row (no b2) straight from s2T
        psmr = pspool.tile([1, 512], F32, tag="ps")
        for cs in range(3):
            nc.tensor.matmul(psmr[:], lhsT=w2c[:, cs:cs + 1],
                             rhs=s2T[:, cs * 512:(cs + 1) * 512],
                             start=(cs == 0), stop=(cs == 2))
        negmu_b = small.tile([1, 512], F32R, tag="negmub")
        nc.scalar.mul(negmu_b[:], psmr[:], -1.0 / C_H)
        # b-side columns: mu from s2T, sumsq from sqb
        psc = pspool.tile([128, 8], F32, tag="ps")
        for c in range(4):
            for cs in range(3):
                nc.tensor.matmul(
                    psc[:, c:c + 1],
                    lhsT=s2T[:, cs * 512 + c * 128:cs * 512 + (c + 1) * 128].bitcast(F32),
                    rhs=w2c[:, cs:cs + 1].bitcast(F32),
                    start=(cs == 0), stop=(cs == 2), skip_group_check=True)
        for c in range(4):
            nc.tensor.matmul(psc[:, 4 + c:5 + c],
                             lhsT=sqb[:, c * 128:(c + 1) * 128].bitcast(F32),
                             rhs=ones_col[:], start=True, stop=True,
                             skip_group_check=True)
        # a-side: columns + row
        psca = pspool.tile([NSH, 2], F32, tag="ps")
        nc.tensor.matmul(psca[:, 0:1], lhsT=apreT[:].bitcast(F32),
                         rhs=ones_col[:],
                         start=True, stop=True, skip_group_check=True)
        nc.tensor.matmul(psca[:, 1:2], lhsT=sqa[:].bitcast(F32),
                         rhs=ones_col[:],
                         start=True, stop=True, skip_group_check=True)
        psra = pspool.tile([1, NSH], F32, tag="ps")
        nc.tensor.matmul(psra[:], lhsT=ones_col_r[:], rhs=apreT[:],
                         start=True, stop=True)
        negmu_a = small.tile([1, NSH], F32R, tag="negmua")
        nc.scalar.mul(negmu_a[:], psra[:], -1.0 / C_H)

        # ============ wave 5: stats chains (ACT/DVE) ============
        negmu_c = small.tile([128, 4], F32, tag="nmcb")
        nc.scalar.mul(negmu_c[:], psc[:, 0:4], -1.0 / C_H)
        e2_c = small.tile([128, 4], F32, tag="e2cb")
        nc.scalar.mul(e2_c[:], psc[:, 4:8], 1.0 / C_H)
        var_c = small.tile([128, 4], F32, tag="varcb")
        nc.vector.tensor_mul(var_c[:], negmu_c[:], negmu_c[:])
        nc.vector.tensor_sub(var_c[:], e2_c[:], var_c[:])
        sd_c = small.tile([128, 4], F32, tag="sdcb")
        nc.scalar.activation(sd_c[:], var_c[:],
                             mybir.ActivationFunctionType.Sqrt,
                             scale=DD * DD, bias=epsD2[:, 0:1])
        rstd_b = small.tile([128, 4], F32, tag="rscb")
        nc.vector.reciprocal(rstd_b[:], sd_c[:])

        negmu_ca = small.tile([NSH, 1], F32, tag="nmca")
        nc.scalar.mul(negmu_ca[:], psca[:, 0:1], -1.0 / C_H)
        e2_ca = small.tile([NSH, 1], F32, tag="e2ca")
        nc.scalar.mul(e2_ca[:], psca[:, 1:2], 1.0 / C_H)
        var_ca = small.tile([NSH, 1], F32, tag="varca")
        nc.vector.tensor_mul(var_ca[:], negmu_ca[:], negmu_ca[:])
        nc.vector.tensor_sub(var_ca[:], e2_ca[:], var_ca[:])
        sd_ca = small.tile([NSH, 1], F32, tag="sdca")
        nc.scalar.activation(sd_ca[:], var_ca[:],
                             mybir.ActivationFunctionType.Sqrt,
                             scale=DD * DD, bias=epsD2[0:NSH, 0:1])
        rstd_a = small.tile([NSH, 1], F32, tag="rsca")
        nc.vector.reciprocal(rstd_a[:], sd_ca[:])

        # ============ wave 6: projections ============
        # pbT[z, (c q)] = sum_s W2W[s, z] s2T[s, (c q)]
        #               + wsum_b[z] * negmu0[(c q)] + b2fix[z]
        ps_pbT = pspool.tile([128, 512], F32, tag="ps")
        nc.tensor.matmul(ps_pbT[:], lhsT=b2fix_row[:], rhs=ones_row_r[:],
                         start=True, stop=False, skip_group_check=True)
        for cs in range(3):
            nc.tensor.matmul(ps_pbT[:], lhsT=W2W[:, cs * 128:(cs + 1) * 128],
                             rhs=s2T[:, cs * 512:(cs + 1) * 512],
                             start=False, stop=False, skip_group_check=True)
        nc.tensor.matmul(ps_pbT[:], lhsT=wsum_bias[0:1, 128:256],
                         rhs=negmu_b[:],
                         start=False, stop=True, skip_group_check=True)
        pbT_s = work.tile([128, 512], F32, tag="pbT")
        nc.scalar.copy(pbT_s[:], ps_pbT[:])

        psp = pspool.tile([NSH, 128], F32, tag="ps")
        nc.tensor.matmul(psp[:], lhsT=apreT[:], rhs=WABT[:, 0:128],
                         start=True, stop=False)
        nc.tensor.matmul(psp[:], lhsT=negmu_a[:], rhs=wsum_bias[0:1, 0:128],
                         start=False, stop=True, skip_group_check=True)
        psbias = pspool.tile([NSH, 128], F32, tag="ps")
        nc.tensor.matmul(psbias[:], lhsT=ones_row_r[0:1, 0:NSH], rhs=bias_dD[:],
                         start=True, stop=True)

        # ============ wave 7: finalize pa / pb ============
        pa_s = work.tile([NSH, 128], F32R, tag="pa_s")
        nc.scalar.mul(pa_s[:], psp[:], rstd_a[:, 0:1])
        nc.vector.tensor_add(pa_s[:], pa_s[:], psbias[:])
        # replicate pa rows onto one partition via SBUF->SBUF DMA:
        # pa_rep2[0, n*256 + c*128 + z] = pa[n, z]
        pa_rep2 = wpool.tile([1, NSH * 256], F32R)
        pa_rep2_v = pa_rep2[:].rearrange("a (n c z) -> a n c z", n=NSH, c=2)
        for c2 in range(2):
            nc.sync.dma_start(pa_rep2_v[:, :, c2:c2 + 1, :], pa_s[:])

        # pb back to [q, z] layout per c-chunk, rstd scale on copy-out
        ps_pb = pspool.tile([128, 512], F32, tag="ps")
        for c in range(4):
            nc.tensor.transpose(ps_pb[:, c * 128:(c + 1) * 128],
                                pbT_s[:, c * 128:(c + 1) * 128], I128[:])
        pb_all = wpool.tile([128, 512], F32R)  # [p, c*128+z] = pb[m=4p+c, z]
        for c in range(4):
            sl = slice(c * 128, (c + 1) * 128)
            if c % 2 == 0:
                nc.scalar.mul(pb_all[:, sl], ps_pb[:, sl], rstd_b[:, c:c + 1])
            else:
                nc.vector.tensor_scalar_mul(pb_all[:, sl], ps_pb[:, sl],
                                            rstd_b[:, c:c + 1])

        # ============ main loop: out[n] = pb_all + rep(pa[n]) ============
        # (the 1/D scale and biases are already folded into pb_all / pa_rep2)
        # Even rows: PE copies pb into psum (I @ pb) + broadcast-add of pa,
        #            then ACT copies psum -> stage.
        # Odd rows:  PE only broadcasts pa into psum; DVE adds pb + psum
        #            straight into the stage (saves the I-matmul + copy).
        # Output DMAs rotate across both HWDGE rings and SWDGE so the
        # descriptor pipelines run in parallel.
        pbr = pb_all[:]
        par = pa_rep2[:]
        dma_engs = (nc.sync, nc.scalar, nc.gpsimd)
        for g in range(NSH // SUP):
            stg = stage_pool.tile([128, SUP * 512], F32, tag="stage")
            for jg in range(0, SUP, MMG):
                pss = []
                for j in range(jg, jg + MMG):
                    ps = psout.tile([128, 512], F32, tag="ps_out")
                    n = g * SUP + j
                    rep = par[0:1, n * 256:(n + 1) * 256]
                    if j % 2 == 0:
                        nc.tensor.matmul(ps[:], lhsT=I128r[:], rhs=pbr,
                                         start=True, stop=False)
                        nc.tensor.matmul(ps[:, 0:256],
                                         lhsT=ones_row_r[0:1, 0:128],
                                         rhs=rep, start=False, stop=False,
                                         skip_group_check=True)
                        nc.tensor.matmul(ps[:, 256:512],
                                         lhsT=ones_row_r[0:1, 0:128],
                                         rhs=rep, start=False, stop=True,
                                         skip_group_check=True)
                    else:
                        nc.tensor.matmul(ps[:, 0:256],
                                         lhsT=ones_row_r[0:1, 0:128],
                                         rhs=rep, start=True, stop=False,
                                         skip_group_check=True)
                        nc.tensor.matmul(ps[:, 256:512],
                                         lhsT=ones_row_r[0:1, 0:128],
                                         rhs=rep, start=False, stop=True,
                                         skip_group_check=True)
                    pss.append(ps)
                for idx, j in enumerate(range(jg, jg + MMG)):
                    dst = stg[:, j * 512:(j + 1) * 512]
                    if j % 2 == 0:
                        nc.scalar.copy(dst, pss[idx][:])
                    else:
                        nc.vector.tensor_add(dst, pbr, pss[idx][:])
            dma_engs[g % 3].dma_start(
                out_r[:, g * SUP:(g + 1) * SUP, :],
                stg[:].rearrange("p (n f) -> p n f", n=SUP),
            )

    nc.compile()
    return nc


_CACHE = {}


def _get_program(bench_loops: int = 1) -> bass.Bass:
    key = ("nc", bench_loops)
    if key not in _CACHE:
        _CACHE[key] = _build_program(bench_loops)
    return _CACHE[key]


def _make_in_maps(inputs: dict) -> list[dict]:
    s1 = np.ascontiguousarray(np.asarray(inputs["s1"], dtype=np.float32))
    s2 = np.ascontiguousarray(np.asarray(inputs["s2"], dtype=np.float32))
    W1 = np.ascontiguousarray(np.asarray(inputs["W1"], dtype=np.float32))
    W2 = np.ascontiguousarray(np.asarray(inputs["W2"], dtype=np.float32))
    Wout = np.ascontiguousarray(np.asarray(inputs["Wout"], dtype=np.float32))
    vecs = np.concatenate([
        np.asarray(inputs[k], dtype=np.float32).reshape(-1)
        for k in ("b1", "b2", "gamma", "beta", "bout")
    ]).reshape(1, -1)
    shared = {
        "s2": s2[0],
        "W1": W1, "W2": W2, "Wout": Wout,
        "vecs": np.ascontiguousarray(vecs),
    }
    in_maps = []
    for i in range(N_CORES):
        m = dict(shared)
        m["s1c"] = np.ascontiguousarray(s1[0, i * NSH:(i + 1) * NSH, :])
        in_maps.append(m)
    return in_maps


def run(inputs: dict, **spmd_kwargs):
    """Build + run on 8 cores; returns (full_output, BassKernelResults)."""
    nc = _get_program()
    in_maps = _make_in_maps(inputs)
    res = run_bass_kernel_spmd(nc, in_maps, list(range(N_CORES)), **spmd_kwargs)
    parts = [res.results[i]["out"] for i in range(N_CORES)]
    full = np.concatenate(parts, axis=0)[None]  # [1, 512, 512, 128]
    return full, res


def kernel(**inputs) -> np.ndarray:
    full, _ = run(inputs)
    return full


if __name__ == "__main__":
    rng = np.random.default_rng(0)
    fake = {
        "s1": rng.standard_normal((1, 512, 384), dtype=np.float32),
        "s2": rng.standard_normal((1, 512, 384), dtype=np.float32),
        "W1": rng.standard_normal((128, 384), dtype=np.float32) / np.sqrt(384),
        "b1": np.zeros(128, np.float32),
        "W2": rng.standard_normal((128, 384), dtype=np.float32) / np.sqrt(384),
        "b2": np.zeros(128, np.float32),
        "gamma": np.ones(128, np.float32),
        "beta": np.zeros(128, np.float32),
        "Wout": rng.standard_normal((128, 256), dtype=np.float32) / np.sqrt(256),
        "bout": np.zeros(128, np.float32),
    }
    o = kernel(**fake)
    print("out", o.shape, o.dtype, float(np.abs(o).mean()))

